# revision 1
# baseline (speedup 1.0000x reference)
"""CAM (channel attention) module kernel for Trainium2, 8-core data-parallel.

Reference computation (per sample, C=512, HW=4096):
    v = x.reshape(C, HW)
    E = v @ v.T                                  # (C, C)
    att = softmax(rowmax(E) - E, axis=-1)        # == softmax(-E) stabilized at rowmin
    o = att @ v                                  # (C, HW)
    o = softmax(o, axis=-1)
    out = x + gamma * o

Sharding: data-parallel over batch B=16 -> 2 samples per NeuronCore, no
cross-core communication. Matmuls run in bf16 on the PE (fp32 PSUM accum),
softmaxes on ACT (exp with fused row-sum accumulation) + DVE.
"""

import sys

if "/opt/trn_rl_repo" not in sys.path:
    sys.path.insert(0, "/opt/trn_rl_repo")

from contextlib import ExitStack

import numpy as np

P = 128
C = 512
HW = 4096
S = 2  # samples per core
CB = C // P  # 4 channel blocks
NB = HW // P  # 32 spatial blocks
NJ = HW // 512  # 8 psum-width chunks
N_CORES = 8

_NC = None


def _build_nc():
    import concourse.bacc as bacc
    import concourse.mybir as mybir
    import concourse.tile as tile
    from concourse.masks import make_identity

    f32 = mybir.dt.float32
    bf16 = mybir.dt.bfloat16
    AF = mybir.ActivationFunctionType
    ALU = mybir.AluOpType
    AX = mybir.AxisListType

    nc = bacc.Bacc("TRN2", target_bir_lowering=False, debug=False, num_devices=N_CORES)
    x = nc.dram_tensor("x", (S, C, HW), f32, kind="ExternalInput").ap()
    gamma = nc.dram_tensor("gamma", (1,), f32, kind="ExternalInput").ap()
    out = nc.dram_tensor("out", (S, C, HW), f32, kind="ExternalOutput").ap()

    with tile.TileContext(nc) as tc, ExitStack() as ctx:
        const = ctx.enter_context(tc.tile_pool(name="const", bufs=1))
        ident = const.tile([P, P], bf16)
        make_identity(nc, ident)
        gamma_sb = const.tile([P, 1], f32)
        nc.sync.dma_start(out=gamma_sb, in_=gamma.to_broadcast((P, 1)))

        xf_pool = ctx.enter_context(tc.tile_pool(name="xf_pool", bufs=5))
        vbf_pool = ctx.enter_context(tc.tile_pool(name="vbf_pool", bufs=5))
        vt_pool = ctx.enter_context(tc.tile_pool(name="vt_pool", bufs=NB + 2))
        att_pool = ctx.enter_context(tc.tile_pool(name="att_pool", bufs=CB + 1))
        attT_pool = ctx.enter_context(tc.tile_pool(name="attT_pool", bufs=CB + 1))
        exp_pool = ctx.enter_context(tc.tile_pool(name="exp_pool", bufs=2))
        small = ctx.enter_context(tc.tile_pool(name="small", bufs=12))
        psum_t = ctx.enter_context(tc.tile_pool(name="psum_t", bufs=2, space="PSUM"))
        psum_e = ctx.enter_context(tc.tile_pool(name="psum_e", bufs=2, space="PSUM"))
        psum_o = ctx.enter_context(tc.tile_pool(name="psum_o", bufs=3, space="PSUM"))

        for s in range(S):
            # ---- load x (f32) and cast to bf16 (GpSimd, otherwise idle) ----
            xf = []
            vb = []
            for i in range(CB):
                xt = xf_pool.tile([P, HW], f32, tag="xf", name=f"xf_{s}_{i}")
                nc.sync.dma_start(out=xt, in_=x[s, i * P : (i + 1) * P, :])
                xf.append(xt)
                vt_ = vbf_pool.tile([P, HW], bf16, tag="vbf", name=f"vb_{s}_{i}")
                nc.gpsimd.tensor_copy(vt_, xt)
                vb.append(vt_)

            # ---- vT[k] (n-part, c-free) via PE transpose of 128x128 blocks ----
            vT = []
            for k in range(NB):
                pt = psum_t.tile([P, C], bf16, tag="pt", name=f"ptv_{s}_{k}")
                for i in range(CB):
                    nc.tensor.transpose(
                        pt[:, i * P : (i + 1) * P],
                        vb[i][:, k * P : (k + 1) * P],
                        ident,
                    )
                st = vt_pool.tile([P, C], bf16, tag="vt", name=f"vT_{s}_{k}")
                if k % 2 == 0:
                    nc.scalar.copy(st, pt)
                else:
                    nc.vector.tensor_copy(st, pt)
                vT.append(st)

            # ---- E = v @ v.T (rows i), then att = exp(rowmin(E) - E) / Z ----
            att = []
            for i in range(CB):
                E = psum_e.tile([P, C], f32, tag="E", name=f"E_{s}_{i}")
                for k in range(NB):
                    nc.tensor.matmul(
                        E,
                        lhsT=vT[k][:, i * P : (i + 1) * P],
                        rhs=vT[k],
                        start=(k == 0),
                        stop=(k == NB - 1),
                    )
                m = small.tile([P, 1], f32, tag="sm", name=f"m_{s}_{i}")
                nc.vector.tensor_reduce(m, E, axis=AX.X, op=ALU.min)
                a = att_pool.tile([P, C], bf16, tag="att", name=f"att_{s}_{i}")
                z1 = small.tile([P, 1], f32, tag="sm", name=f"z1_{s}_{i}")
                nc.scalar.activation(a, E, AF.Exp, bias=m, scale=-1.0, accum_out=z1)
                r1 = small.tile([P, 1], f32, tag="sm", name=f"r1_{s}_{i}")
                nc.vector.reciprocal(r1, z1)
                nc.vector.tensor_scalar_mul(a, a, r1)
                att.append(a)

            # ---- attT[j] (col-part, row-free) via PE transpose ----
            attT = []
            for j in range(CB):
                pt = psum_t.tile([P, C], bf16, tag="pt", name=f"pta_{s}_{j}")
                for i in range(CB):
                    nc.tensor.transpose(
                        pt[:, i * P : (i + 1) * P],
                        att[i][:, j * P : (j + 1) * P],
                        ident,
                    )
                st = attT_pool.tile([P, C], bf16, tag="attT", name=f"attT_{s}_{j}")
                if j % 2 == 0:
                    nc.vector.tensor_copy(st, pt)
                else:
                    nc.scalar.copy(st, pt)
                attT.append(st)

            # ---- o = att @ v, second softmax over HW, final out = x + g*o ----
            for i in range(CB):
                er = exp_pool.tile([P, HW], bf16, tag="er", name=f"er_{s}_{i}")
                z2p = small.tile([P, NJ], f32, tag="z2p", name=f"z2p_{s}_{i}")
                for nj in range(NJ):
                    o2 = psum_o.tile([P, 512], f32, tag="o2", name=f"o2_{s}_{i}_{nj}")
                    for kc in range(CB):
                        nc.tensor.matmul(
                            o2,
                            lhsT=attT[kc][:, i * P : (i + 1) * P],
                            rhs=vb[kc][:, nj * 512 : (nj + 1) * 512],
                            start=(kc == 0),
                            stop=(kc == CB - 1),
                        )
                    nc.scalar.activation(
                        er[:, nj * 512 : (nj + 1) * 512],
                        o2,
                        AF.Exp,
                        accum_out=z2p[:, nj : nj + 1],
                    )
                z2 = small.tile([P, 1], f32, tag="sm", name=f"z2_{s}_{i}")
                nc.vector.reduce_sum(z2, z2p, axis=AX.X)
                r2 = small.tile([P, 1], f32, tag="sm", name=f"r2_{s}_{i}")
                nc.vector.reciprocal(r2, z2)
                gz = small.tile([P, 1], f32, tag="sm", name=f"gz_{s}_{i}")
                nc.vector.tensor_scalar_mul(gz, r2, gamma_sb)
                nc.vector.scalar_tensor_tensor(
                    out=xf[i],
                    in0=er,
                    scalar=gz,
                    in1=xf[i],
                    op0=ALU.mult,
                    op1=ALU.add,
                )
                nc.sync.dma_start(out=out[s, i * P : (i + 1) * P, :], in_=xf[i])

    nc.compile()
    return nc


def get_nc():
    global _NC
    if _NC is None:
        _NC = _build_nc()
    return _NC


def kernel(x: np.ndarray, gamma: np.ndarray) -> np.ndarray:
    from concourse.bass_utils import run_bass_kernel_spmd

    B, Cx, H, W = x.shape
    assert (B, Cx, H * W) == (16, C, HW), (B, Cx, H, W)
    nc = get_nc()
    xs = np.ascontiguousarray(np.asarray(x, dtype=np.float32)).reshape(B, Cx, H * W)
    g = np.ascontiguousarray(np.asarray(gamma, dtype=np.float32)).reshape(1)
    in_maps = [
        {"x": xs[S * c : S * (c + 1)], "gamma": g} for c in range(N_CORES)
    ]
    res = run_bass_kernel_spmd(nc, in_maps, core_ids=list(range(N_CORES)))
    out = np.concatenate([res.results[c]["out"] for c in range(N_CORES)], axis=0)
    return out.reshape(B, Cx, H, W).astype(np.float32)
